# revision 1
# baseline (speedup 1.0000x reference)
"""ChunkedAttention (nn_ChunkedAttention_43568148251092) Trainium2 kernel.

Full inputs q/k/v: [1, 4096, 16, 128] fp32. Shards the 16 heads across the
8 NeuronCores (2 heads per core, pure head parallelism — no collectives),
runs a Bass/Tile attention kernel per core, and concatenates the results.

Per-head pipeline on each core (S=4096 tokens, D=128):
  - int8 quant-dequant of K and V per token, trunc-toward-zero exactly as the
    reference (RNE int convert + compare fixup; no native trunc on DVE).
    Kint kept as fp16 integers (exact: |int| <= 127), per-token kscale kept
    fp32 and folded into the softmax exp via the ACT per-partition scale.
  - Q cast to fp16 (single pass; the output error is dominated by the bf16
    P'/Vdq storage, measured equal to a bf16 hi+lo split); Q and Kint
    transposed to [d, s] via PE transpose (Kint exact in fp16).
  - S^T[k,q] = KintT.T @ QT in PSUM fp32 (hi_lo=True option adds a lo pass).
  - P'[k,q] = exp(kscale/sqrt(D) * S^T - 40) via ScalarE (bias keeps the
    fp32/bf16 range safe without a row-max pass; scores are ~N(0, sqrt(128))).
  - out[q, 0:128|denom] = sum_kt P'_kt.T @ [Vdq | ones] accumulated in PSUM;
    the appended ones-column yields the softmax denominator for free.
  - out = out[:, :128] * (1/denom) per partition, DMA to DRAM.
"""

import math

import numpy as np

import concourse.bass as bass
import concourse.mybir as mybir
import concourse.tile as tile
from concourse import bacc
from concourse.bass_utils import run_bass_kernel_spmd
from concourse.masks import make_identity

F32 = mybir.dt.float32
BF16 = mybir.dt.bfloat16
FP16 = mybir.dt.float16
I32 = mybir.dt.int32
AX = mybir.AxisListType.X
OP = mybir.AluOpType
EXP = mybir.ActivationFunctionType.Exp

_S = 4096
_H_TOTAL = 16
_D = 128
_N_CORES = 8
_H = _H_TOTAL // _N_CORES  # heads per core

_NC_CACHE = {}


def _bcast3(ap2, n):
    """[128, J] AP -> [128, J, n] broadcast AP (inner stride 0)."""
    return bass.AP(tensor=ap2.tensor, offset=ap2.offset, ap=[*ap2.ap, [0, n]])


def _trunc(nc, pool, x, out, out_slice=None, scale_bcast=None, eng=None):
    """Exact trunc-toward-zero of fp32 tile x (|x| <= ~127) into `out`.

    DVE has no trunc/floor/mod and its int converts round-to-nearest-even,
    so: r = RNE(x); fix = clamp(1e38 * x * [x*(r-x) > 0], -1, 1); r - fix.
    If scale_bcast is given, writes trunc(x)*scale instead (V dequant).
    """
    e = eng if eng is not None else nc.vector
    shp = list(x.shape)
    ri = pool.tile(shp, I32, tag="t_ri")
    e.tensor_copy(ri[:], x[:])
    rf = pool.tile(shp, F32, tag="t_rf")
    e.tensor_copy(rf[:], ri[:])
    d = pool.tile(shp, F32, tag="t_d")
    e.tensor_tensor(d[:], rf[:], x[:], op=OP.subtract)
    e.tensor_tensor(d[:], d[:], x[:], op=OP.mult)
    e.tensor_scalar(d[:], d[:], 0.0, None, op0=OP.is_gt)
    e.tensor_tensor(d[:], d[:], x[:], op=OP.mult)
    e.tensor_scalar(d[:], d[:], 1e38, 1.0, op0=OP.mult, op1=OP.min)
    e.tensor_scalar(d[:], d[:], -1.0, None, op0=OP.max)
    dst = out[out_slice] if out_slice is not None else out[:]
    if scale_bcast is None:
        e.tensor_tensor(dst, rf[:], d[:], op=OP.subtract)
    else:
        tr = pool.tile(shp, F32, tag="t_tr")
        e.tensor_tensor(tr[:], rf[:], d[:], op=OP.subtract)
        e.tensor_tensor(dst, tr[:], scale_bcast, op=OP.mult)


def _build_nc(S=_S, H=_H, D=_D, qc_cols=1024, c_bias=40.0, hi_lo=False,
              pp_bufs=None, qk_dt=FP16, trunc_eng=None, tcopy_eng="any",
              ld_bufs=8, tmp_bufs=4, b16_bufs=6, psT_bufs=2, psS_bufs=2):
    assert D == 128 and S % 512 == 0 and qc_cols % 512 == 0
    n_kt = S // 128
    n_grp = S // 512
    n_qc = S // qc_cols
    qt_per_qc = qc_cols // 128
    if pp_bufs is None:
        pp_bufs = n_kt + 4

    nc = bacc.Bacc("TRN2")
    q_d = nc.dram_tensor("q", [S, H, D], F32, kind="ExternalInput")
    k_d = nc.dram_tensor("k", [S, H, D], F32, kind="ExternalInput")
    v_d = nc.dram_tensor("v", [S, H, D], F32, kind="ExternalInput")
    o_d = nc.dram_tensor("o", [S, H, D], F32, kind="ExternalOutput")

    with tile.TileContext(nc) as tc:
        with (
            tc.tile_pool(name="const", bufs=1) as constp,
            tc.tile_pool(name="big", bufs=1) as bigp,
            tc.tile_pool(name="ld", bufs=ld_bufs) as ldp,
            tc.tile_pool(name="tmp", bufs=tmp_bufs) as tmpp,
            tc.tile_pool(name="b16", bufs=b16_bufs) as b16p,
            tc.tile_pool(name="small", bufs=6) as smallp,
            tc.tile_pool(name="pp", bufs=pp_bufs) as ppool,
            tc.tile_pool(name="outp", bufs=4) as outp,
            tc.tile_pool(name="psT", bufs=psT_bufs, space="PSUM") as psT,
            tc.tile_pool(name="psS", bufs=psS_bufs, space="PSUM") as psS,
            tc.tile_pool(name="psO", bufs=2, space="PSUM") as psO,
        ):
            ident32 = constp.tile([128, 128], F32)
            make_identity(nc, ident32[:])
            ident16 = constp.tile([128, 128], qk_dt)
            nc.vector.tensor_copy(ident16[:], ident32[:])
            ceng = nc.any if tcopy_eng == "any" else getattr(nc, tcopy_eng)
            if not hasattr(ceng, "tensor_copy"):
                ceng = nc.vector
            bias_t = constp.tile([128, 1], F32)
            nc.vector.memset(bias_t[:], -c_bias)

            for h in range(H):
                KT = bigp.tile([128, S], qk_dt, tag="KT")
                QThi = bigp.tile([128, S], qk_dt, tag="QThi")
                if hi_lo:
                    QTlo = bigp.tile([128, S], qk_dt, tag="QTlo")
                Vext = bigp.tile([128, n_kt, 132], BF16, tag="Vext")
                kscales = bigp.tile([128, n_kt], F32, tag="kscales")
                nc.vector.memset(Vext[:], 0.0)

                for g in range(n_grp):
                    s0 = g * 512
                    # ---- K: quantize to Kint (bf16 ints) + kscale ----
                    kf = ldp.tile([128, 4, 128], F32, tag="ld")
                    nc.sync.dma_start(
                        out=kf[:],
                        in_=k_d[s0:s0 + 512, h, :].rearrange(
                            "(j p) d -> p j d", p=128))
                    am = smallp.tile([128, 4], F32, tag="am")
                    nc.vector.reduce_max(am[:], kf[:], axis=AX,
                                         apply_absolute_value=True)
                    sc = smallp.tile([128, 4], F32, tag="sc")
                    nc.vector.tensor_scalar(sc[:], am[:], 1e-8, 1.0 / 127.0,
                                            op0=OP.max, op1=OP.mult)
                    nc.vector.tensor_scalar(
                        kscales[:, 4 * g:4 * g + 4], sc[:],
                        1.0 / math.sqrt(128.0), None, op0=OP.mult)
                    rc = smallp.tile([128, 4], F32, tag="rc")
                    nc.vector.reciprocal(rc[:], sc[:])
                    x = tmpp.tile([128, 4, 128], F32, tag="x")
                    nc.vector.tensor_tensor(x[:], kf[:], _bcast3(rc[:], 128),
                                            op=OP.mult)
                    kint = b16p.tile([128, 4, 128], qk_dt, tag="i16")
                    _trunc(nc, tmpp, x, kint, eng=trunc_eng and nc.gpsimd)
                    for j in range(4):
                        pst = psT.tile([128, 128], qk_dt, tag="pst")
                        nc.tensor.transpose(pst[:], kint[:, j, :], ident16[:])
                        kt_i = 4 * g + j
                        ceng.tensor_copy(
                            KT[:, kt_i * 128:(kt_i + 1) * 128], pst[:])

                    # ---- V: quantize + dequant into Vext (+ ones col) ----
                    vf = ldp.tile([128, 4, 128], F32, tag="ld")
                    nc.sync.dma_start(
                        out=vf[:],
                        in_=v_d[s0:s0 + 512, h, :].rearrange(
                            "(j p) d -> p j d", p=128))
                    am2 = smallp.tile([128, 4], F32, tag="am")
                    nc.vector.reduce_max(am2[:], vf[:], axis=AX,
                                         apply_absolute_value=True)
                    sc2 = smallp.tile([128, 4], F32, tag="sc")
                    nc.vector.tensor_scalar(sc2[:], am2[:], 1e-8, 1.0 / 127.0,
                                            op0=OP.max, op1=OP.mult)
                    rc2 = smallp.tile([128, 4], F32, tag="rc")
                    nc.vector.reciprocal(rc2[:], sc2[:])
                    xv_ = tmpp.tile([128, 4, 128], F32, tag="x")
                    nc.vector.tensor_tensor(xv_[:], vf[:], _bcast3(rc2[:], 128),
                                            op=OP.mult)
                    _trunc(nc, tmpp, xv_, Vext,
                           out_slice=(slice(None), slice(4 * g, 4 * g + 4),
                                      slice(0, 128)),
                           scale_bcast=_bcast3(sc2[:], 128),
                           eng=trunc_eng and nc.gpsimd)
                    nc.vector.memset(Vext[:, 4 * g:4 * g + 4, 128:129], 1.0)

                    # ---- Q: hi/lo split + transpose ----
                    qf = ldp.tile([128, 4, 128], F32, tag="ld")
                    nc.sync.dma_start(
                        out=qf[:],
                        in_=q_d[s0:s0 + 512, h, :].rearrange(
                            "(j p) d -> p j d", p=128))
                    qhi = b16p.tile([128, 4, 128], qk_dt, tag="i16")
                    nc.vector.tensor_copy(qhi[:], qf[:])
                    if hi_lo:
                        qhw = tmpp.tile([128, 4, 128], F32, tag="qhw")
                        nc.vector.tensor_copy(qhw[:], qhi[:])
                        qlo = b16p.tile([128, 4, 128], qk_dt, tag="i16")
                        nc.vector.tensor_tensor(qlo[:], qf[:], qhw[:],
                                                op=OP.subtract)
                    for j in range(4):
                        kt_i = 4 * g + j
                        pst = psT.tile([128, 128], qk_dt, tag="pst")
                        nc.tensor.transpose(pst[:], qhi[:, j, :], ident16[:])
                        ceng.tensor_copy(
                            QThi[:, kt_i * 128:(kt_i + 1) * 128], pst[:])
                        if hi_lo:
                            pst2 = psT.tile([128, 128], qk_dt, tag="pst")
                            nc.tensor.transpose(pst2[:], qlo[:, j, :],
                                                ident16[:])
                            ceng.tensor_copy(
                                QTlo[:, kt_i * 128:(kt_i + 1) * 128], pst2[:])

                # ---------- main attention loops ----------
                for qc in range(n_qc):
                    p_tiles = []
                    for kt in range(n_kt):
                        sps = psS.tile([128, qc_cols], F32, tag="sps")
                        w = KT[:, kt * 128:(kt + 1) * 128]
                        for half in range(qc_cols // 512):
                            c0 = qc * qc_cols + half * 512
                            dst = sps[:, half * 512:(half + 1) * 512]
                            nc.tensor.matmul(dst, w, QThi[:, c0:c0 + 512],
                                             start=True, stop=not hi_lo)
                            if hi_lo:
                                nc.tensor.matmul(dst, w, QTlo[:, c0:c0 + 512],
                                                 start=False, stop=True)
                        pt = ppool.tile([128, qc_cols], BF16, tag="pp")
                        nc.scalar.activation(pt[:], sps[:], EXP,
                                             bias=bias_t[:],
                                             scale=kscales[:, kt:kt + 1])
                        p_tiles.append(pt)
                    for qt in range(qt_per_qc):
                        ops_ = psO.tile([128, 132], F32, tag="ops")
                        for kt in range(n_kt):
                            nc.tensor.matmul(
                                ops_[:],
                                p_tiles[kt][:, qt * 128:(qt + 1) * 128],
                                Vext[:, kt, :],
                                start=(kt == 0), stop=(kt == n_kt - 1))
                        rcp = smallp.tile([128, 1], F32, tag="rcp")
                        nc.vector.reciprocal(rcp[:], ops_[:, 128:129])
                        ot = outp.tile([128, 128], F32, tag="ot")
                        nc.vector.tensor_scalar(ot[:], ops_[:, 0:128], rcp[:],
                                                None, op0=OP.mult)
                        q0 = qc * qc_cols + qt * 128
                        nc.sync.dma_start(out=o_d[q0:q0 + 128, h, :],
                                          in_=ot[:])

    nc.compile()
    return nc


def get_nc(**kwargs):
    key = tuple(sorted(kwargs.items()))
    if key not in _NC_CACHE:
        _NC_CACHE[key] = _build_nc(**kwargs)
    return _NC_CACHE[key]


def kernel(q, k, v, _trace=False, _trace_cores=None, _nc_kwargs=None):
    """Full-input entry point: q/k/v [1, 4096, 16, 128] fp32 -> same shape."""
    assert q.shape == (1, _S, _H_TOTAL, _D), q.shape
    nc = get_nc(**(_nc_kwargs or {}))
    in_maps = []
    for c in range(_N_CORES):
        hs = slice(c * _H, (c + 1) * _H)
        in_maps.append({
            "q": np.ascontiguousarray(q[0, :, hs, :], dtype=np.float32),
            "k": np.ascontiguousarray(k[0, :, hs, :], dtype=np.float32),
            "v": np.ascontiguousarray(v[0, :, hs, :], dtype=np.float32),
        })
    # The axon-tunneled device occasionally reports a transient
    # NRT_EXEC_UNIT_UNRECOVERABLE on the first execution; a retry succeeds.
    last_err = None
    for attempt in range(3):
        try:
            res = run_bass_kernel_spmd(nc, in_maps,
                                       core_ids=list(range(_N_CORES)),
                                       trace=_trace, trace_cores=_trace_cores)
            break
        except Exception as e:  # noqa: BLE001
            last_err = e
            time.sleep(2.0 * (attempt + 1))
    else:
        raise last_err
    out = np.concatenate([res.results[c]["o"] for c in range(_N_CORES)],
                         axis=1)[None]
    out = np.ascontiguousarray(out, dtype=np.float32)
    if _trace:
        return out, res
    return out



# revision 7
# speedup vs baseline: 1.3117x; 1.3117x over previous
"""ChunkedAttention (nn_ChunkedAttention_43568148251092) Trainium2 kernel.

Full inputs q/k/v: [1, 4096, 16, 128] fp32. Shards the 16 heads across the
8 NeuronCores (2 heads per core, pure head parallelism - no collectives),
runs a Bass/Tile attention kernel per core, and concatenates the results.

Per-core design (S=4096, D=128, 2 heads), engine-balanced for the TRN2
cost model (ACT exp is the pacer at ~251us; PE ~229us; DVE ~195us):

  - K/V int8 quant-dequant emulation per token (trunc-toward-zero via the
    RNE-convert of x - 0.49999997*sign(x); |x| <= 127 so no clamp needed).
    K is dequantized straight into fp16 (scale folded in), so the exp
    scale/bias are CONSTANT and one ACT instruction can span several
    score tiles.
  - Q cast to fp16; K/Q transposed to [d, s] via PE transposes (psum
    staging, DVE copies out).
  - Scores S^T[k, q] = KdqT.T @ QT accumulate in a 6-bank PSUM ring of
    [128, 3, 512] tiles; ACT computes P' = exp(s/sqrt(D) - 40) over all
    1536 columns of a tile in one instruction, writing bf16 into per-half-
    chunk P' buffers in SBUF.
  - PV: out[q, 0:128|denom] = sum_kt P'_kt.T @ [Vdq | ones] accumulated in
    a packed 1-bank psum pair; the ones-column yields the softmax
    denominator. DVE normalizes, SP DMAs out.
  - Software pipelining: PV matmuls of the previous chunk and the next
    head's transposes are interleaved between score-tile matmuls in PE
    program order; the next head's DVE quant work is doled out per chunk.
"""

import math
import time

import numpy as np

import concourse.bass as bass
import concourse.mybir as mybir
import concourse.tile as tile
from concourse import bacc
from concourse.bass_utils import run_bass_kernel_spmd
from concourse.masks import make_identity

F32 = mybir.dt.float32
BF16 = mybir.dt.bfloat16
FP16 = mybir.dt.float16
I32 = mybir.dt.int32
AX = mybir.AxisListType.X
OP = mybir.AluOpType
EXP = mybir.ActivationFunctionType.Exp

_S = 4096
_H_TOTAL = 16
_D = 128
_N_CORES = 8
_H = _H_TOTAL // _N_CORES  # heads per core

_N_KT = _S // 128          # 32 k tiles per head
_N_GRP = _S // 512         # 8 token groups per head
_N_CHUNK = 4               # q chunks of 1024 per head
_SLOTS_PER_HALF = _N_KT    # 32 score slots of 512 q-cols per chunk half

_SCALE = 1.0 / math.sqrt(128.0)
_BIAS = -40.0

_NC_CACHE = {}


def _bcast3(ap2, n):
    """[128, J] AP -> [128, J, n] broadcast AP (inner stride 0)."""
    return bass.AP(tensor=ap2.tensor, offset=ap2.offset, ap=[*ap2.ap, [0, n]])


def _build_nc(H=_H, pv_cols=129):
    nc = bacc.Bacc("TRN2")
    q_d = nc.dram_tensor("q", [_S, H, _D], F32, kind="ExternalInput")
    k_d = nc.dram_tensor("k", [_S, H, _D], F32, kind="ExternalInput")
    v_d = nc.dram_tensor("v", [_S, H, _D], F32, kind="ExternalInput")
    o_d = nc.dram_tensor("o", [_S, H, _D], F32, kind="ExternalOutput")

    with tile.TileContext(nc) as tc:
        with (
            tc.tile_pool(name="const", bufs=1) as constp,
            tc.tile_pool(name="big", bufs=2) as bigp,
            tc.tile_pool(name="ld", bufs=4) as ldp,
            tc.tile_pool(name="tmp", bufs=2) as tmpp,
            tc.tile_pool(name="b16", bufs=3) as b16p,
            tc.tile_pool(name="small", bufs=2) as smallp,
            tc.tile_pool(name="pp", bufs=3) as ppool,
            tc.tile_pool(name="outp", bufs=3) as outp,
            tc.tile_pool(name="psS", bufs=2, space="PSUM") as psS,
            tc.tile_pool(name="psX", bufs=1, space="PSUM") as psX,
        ):
            ident32 = constp.tile([128, 128], F32)
            make_identity(nc, ident32[:])
            ident16 = constp.tile([128, 128], FP16)
            nc.vector.tensor_copy(ident16[:], ident32[:])
            bias_t = constp.tile([128, 1], F32)
            nc.vector.memset(bias_t[:], _BIAS)

            # Persistent per-head tensors (2 bufs: current + next head).
            def head_tensors():
                KT = bigp.tile([128, _S], FP16, tag="KT")
                QT = bigp.tile([128, _S], FP16, tag="QT")
                Vext = bigp.tile([128, _N_KT, 132], BF16, tag="Vext")
                return KT, QT, Vext

            # Packed psum: 2 PV accumulators in one bank + 4 transpose
            # staging slots in another.
            ops_t = psX.tile([128, 2, 132], F32, tag="ops")
            pst_t = psX.tile([128, 4, 128], FP16, tag="pst")
            pst_ring = [0]
            heads = [head_tensors()]

            def quant_unit(h, which, g, dst):
                """Emit one DVE quant unit: K/Q/V for token group g of head
                h. dst = (KT, QT, Vext) tensors for head h. Returns list of
                PE transpose closures (empty for V)."""
                KT, QT, Vext = dst
                s0 = g * 512
                src = {"K": k_d, "Q": q_d, "V": v_d}[which]
                ld = ldp.tile([128, 4, 128], F32, tag="ld", name=f"ld{which}")
                nc.sync.dma_start(
                    out=ld[:],
                    in_=src[s0:s0 + 512, h % H, :].rearrange(
                        "(j p) d -> p j d", p=128))
                if which == "Q":
                    q16 = b16p.tile([128, 4, 128], FP16, tag="i16", name="q16")
                    nc.vector.tensor_copy(q16[:], ld[:])
                    return [_transpose_closure(q16, j, QT, g * 4 + j)
                            for j in range(4)]
                # K/V: per-token absmax -> scale -> x = v/scale ->
                # trunc-toward-zero -> dequant.
                am = smallp.tile([128, 4], F32, tag="am")
                nc.vector.reduce_max(am[:], ld[:], axis=AX,
                                     apply_absolute_value=True)
                sc = smallp.tile([128, 4], F32, tag="sc")
                nc.vector.tensor_scalar(sc[:], am[:], 1e-8, 1.0 / 127.0,
                                        op0=OP.max, op1=OP.mult)
                rc = smallp.tile([128, 4], F32, tag="rc")
                nc.vector.reciprocal(rc[:], sc[:])
                x = tmpp.tile([128, 4, 128], F32, tag="x")
                nc.vector.tensor_tensor(x[:], ld[:], _bcast3(rc[:], 128),
                                        op=OP.mult)
                # c = 0.49999997 - 0.99999994*(x>=0)  (= -+0.49999997)
                c = tmpp.tile([128, 4, 128], F32, tag="c")
                nc.vector.tensor_scalar(c[:], x[:], 0.0, None, op0=OP.is_ge)
                nc.vector.tensor_scalar(c[:], c[:], -0.99999994, 0.49999997,
                                        op0=OP.mult, op1=OP.add)
                nc.vector.tensor_tensor(x[:], x[:], c[:], op=OP.add)
                yi = tmpp.tile([128, 4, 128], I32, tag="yi")
                nc.vector.tensor_copy(yi[:], x[:])  # RNE int convert
                if which == "K":
                    kdq = b16p.tile([128, 4, 128], FP16, tag="i16", name="kdq")
                    nc.vector.tensor_tensor(kdq[:], yi[:], _bcast3(sc[:], 128),
                                            op=OP.mult)
                    return [_transpose_closure(kdq, j, KT, g * 4 + j)
                            for j in range(4)]
                nc.vector.tensor_tensor(
                    Vext[:, 4 * g:4 * g + 4, 0:128], yi[:],
                    _bcast3(sc[:], 128), op=OP.mult)
                return []

            def _transpose_closure(src16, j, dstT, kt_i):
                def run():
                    tj = pst_ring[0]
                    pst_ring[0] = (tj + 1) % 4
                    nc.tensor.transpose(pst_t[:, tj, :], src16[:, j, :],
                                        ident16[:])
                    nc.vector.tensor_copy(
                        dstT[:, kt_i * 128:(kt_i + 1) * 128], pst_t[:, tj, :])
                return run

            def head_setup_units(h, dst, prologue=False):
                """Ordered quant units for head h. The prologue order makes
                chunk 0 runnable as early as possible (Q0/Q1 early, K
                progressive); the steady-state order frontloads K."""
                KT, QT, Vext = dst
                nc.vector.memset(Vext[:, :, 128:132], 0.0)
                nc.vector.memset(Vext[:, :, 128:129], 1.0)
                if prologue:
                    order = ([("K", 0), ("Q", 0), ("Q", 1)]
                             + [("K", g) for g in range(1, 8)]
                             + [("Q", 2), ("Q", 3), ("V", 0), ("V", 1),
                                ("Q", 4), ("Q", 5), ("V", 2), ("V", 3),
                                ("Q", 6), ("Q", 7), ("V", 4), ("V", 5),
                                ("V", 6), ("V", 7)])
                else:
                    order = ([("K", g) for g in range(8)]
                             + [("Q", g) for g in range(8)]
                             + [("V", g) for g in range(8)])
                yield from order

            def pv_stream(h, c, halves, Vext):
                """Yield PE closures for the PV of chunk (h, c); drains
                (DVE normalize + SP store) are emitted inline after each
                qt's last matmul."""
                for qt in range(8):
                    ph = halves[qt // 4]
                    q128 = (qt % 4) * 128
                    slot = qt % 2
                    for kt in range(_N_KT):
                        def mm(kt=kt, ph=ph, q128=q128, slot=slot, qt=qt):
                            nc.tensor.matmul(
                                ops_t[:, slot, 0:pv_cols],
                                ph[:, kt, q128:q128 + 128],
                                Vext[:, kt, 0:pv_cols],
                                start=(kt == 0), stop=(kt == _N_KT - 1))
                            if kt == _N_KT - 1:
                                rcp = smallp.tile([128, 1], F32, tag="rcp")
                                nc.vector.reciprocal(rcp[:],
                                                     ops_t[:, slot, 128:129])
                                ot = outp.tile([128, 128], F32, tag="ot")
                                nc.vector.tensor_scalar(
                                    ot[:], ops_t[:, slot, 0:128], rcp[:],
                                    None, op0=OP.mult)
                                q0 = c * 1024 + qt * 128
                                nc.sync.dma_start(
                                    out=o_d[q0:q0 + 128, h % H, :], in_=ot[:])
                        yield mm

            # ---------------- global schedule ----------------
            chunk_list = [(h, c) for h in range(H) for c in range(_N_CHUNK)]
            head_ts = {0: heads[0]}

            # Prologue: head 0 setup (K first; transposes inline).
            for which, g in head_setup_units(0, head_ts[0]):
                for tr in quant_unit(0, which, g, head_ts[0]):
                    tr()

            pending_tr = []      # PE transpose closures (next head's K/Q)
            pv_halves = {}       # (h, c) -> [half0_tile, half1_tile]
            units_next = []

            for idx, (h, c) in enumerate(chunk_list):
                KT, QT, Vext = head_ts[h]
                # Next head's DVE setup work: 6 units doled out per chunk,
                # interleaved into the tile loop (so PV drains on DVE are
                # reached promptly).
                if h + 1 < H and c == 0:
                    head_ts[h + 1] = head_tensors()
                    units_next = list(head_setup_units(h + 1, head_ts[h + 1]))
                units_now = units_next[6 * c:6 * c + 6] if h + 1 < H else []

                # Background PE work: transposes due + PV of previous chunk.
                bg = list(pending_tr)
                pending_tr = []
                if idx > 0:
                    ph_, pc_ = chunk_list[idx - 1]
                    bg.extend(pv_stream(ph_, pc_,
                                        pv_halves.pop((ph_, pc_)),
                                        head_ts[ph_][2]))
                n_bg = len(bg)
                n_units = len(units_now)
                bg_done = 0
                units_done = 0

                halves = []
                n_tiles = 22  # 2 halves x (10 full + 1 two-slot)
                tile_i = 0
                for qh in range(2):
                    half = ppool.tile([128, _N_KT, 512], BF16, tag="pp",
                                      name="phalf")
                    halves.append(half)
                    qcol = c * 1024 + qh * 512
                    kt = 0
                    for tsz in [3] * 10 + [2]:
                        sps = psS.tile([128, 3, 512], F32, tag="sps")
                        for j in range(tsz):
                            nc.tensor.matmul(
                                sps[:, j, :],
                                KT[:, (kt + j) * 128:(kt + j + 1) * 128],
                                QT[:, qcol:qcol + 512],
                                start=True, stop=True)
                        nc.scalar.activation(
                            half[:, kt:kt + tsz, :].rearrange(
                                "p a b -> p (a b)"),
                            sps[:, 0:tsz, :].rearrange("p a b -> p (a b)"),
                            EXP, bias=bias_t[:], scale=_SCALE)
                        kt += tsz
                        tile_i += 1
                        # interleave background PE work + DVE setup units
                        while bg_done < (n_bg * tile_i) // n_tiles:
                            bg[bg_done]()
                            bg_done += 1
                        while units_done < (n_units * tile_i) // n_tiles:
                            which, g = units_now[units_done]
                            pending_tr.extend(
                                quant_unit(h + 1, which, g, head_ts[h + 1]))
                            units_done += 1
                pv_halves[(h, c)] = halves
                while bg_done < n_bg:
                    bg[bg_done]()
                    bg_done += 1

            # Trailing PV for the last chunk.
            h_, c_ = chunk_list[-1]
            for fn in pv_stream(h_, c_, pv_halves.pop((h_, c_)),
                                head_ts[h_][2]):
                fn()

    nc.compile()
    return nc


def get_nc(**kwargs):
    key = tuple(sorted(kwargs.items()))
    if key not in _NC_CACHE:
        _NC_CACHE[key] = _build_nc(**kwargs)
    return _NC_CACHE[key]


def kernel(q, k, v, _trace=False, _trace_cores=None, _nc_kwargs=None):
    """Full-input entry point: q/k/v [1, 4096, 16, 128] fp32 -> same shape."""
    assert q.shape == (1, _S, _H_TOTAL, _D), q.shape
    nc = get_nc(**(_nc_kwargs or {}))
    in_maps = []
    for cid in range(_N_CORES):
        hs = slice(cid * _H, (cid + 1) * _H)
        in_maps.append({
            "q": np.ascontiguousarray(q[0, :, hs, :], dtype=np.float32),
            "k": np.ascontiguousarray(k[0, :, hs, :], dtype=np.float32),
            "v": np.ascontiguousarray(v[0, :, hs, :], dtype=np.float32),
        })
    # The axon-tunneled device occasionally reports a transient
    # NRT_EXEC_UNIT_UNRECOVERABLE on the first execution; a retry succeeds.
    last_err = None
    for attempt in range(3):
        try:
            res = run_bass_kernel_spmd(nc, in_maps,
                                       core_ids=list(range(_N_CORES)),
                                       trace=_trace, trace_cores=_trace_cores)
            break
        except Exception as e:  # noqa: BLE001
            last_err = e
            time.sleep(2.0 * (attempt + 1))
    else:
        raise last_err
    out = np.concatenate([res.results[cid]["o"] for cid in range(_N_CORES)],
                         axis=1)[None]
    out = np.ascontiguousarray(out, dtype=np.float32)
    if _trace:
        return out, res
    return out
